# revision 4
# baseline (speedup 1.0000x reference)
"""Multi-head attention (B=2, L=2048, H=16, D=64) on 8 TRN2 NeuronCores.

Sharding: core = (batch b, head-group hg); 2 batches x 4 groups of 4 heads.
Each core computes, for its batch and its 4 heads:
    Q^T/K^T = W^T x^T           (per-head slices, d on partitions)
    V       = x W_v             (j on partitions, + ones column for denom)
    S^T     = K^T.T Q^T         (j on partitions, i free)
    P'      = exp(S^T/8)        (un-normalized)
    O'^T    = [V|1].T P'        (row 64 = softmax denominator)
    O^T     = O'[0:64] * (1/O'[64])
    out^T  += Wo_rows^T O^T     (partial over head-group rows of Wo)
Host sums the 4 partials per batch, transposes, adds bo.

All matmuls run in float32r (TF32-like, full PE rate at N>=256).
"""

import sys

try:
    import concourse.bass as bass  # noqa: F401
except ImportError:  # pragma: no cover - path fallback
    sys.path.insert(0, "/opt/trn_rl_repo")

import numpy as np
import concourse.bass as bass
import concourse.mybir as mybir
import concourse.tile as tile
from concourse import bacc
from concourse.bass_utils import run_bass_kernel_spmd

F32 = mybir.dt.float32
F32R = mybir.dt.float32r
AF = mybir.ActivationFunctionType

B = 2
L = 2048          # sequence length
C = 1024          # model dim
H_LOC = 4         # heads per core
D = 64            # head dim
HD = H_LOC * D    # 256 = local head-group width
KT = C // 128     # 8 k-tiles over the model dim
SCALE2 = float(D) ** -0.5  # 1/8, applied once inside exp

_cache = {}


def _build():
    nc = bacc.Bacc("TRN2", target_bir_lowering=False, debug=False, num_devices=8)

    xT = nc.declare_dram_parameter("xT", [C, L], F32R, isOutput=False)
    wq = nc.declare_dram_parameter("wq", [C, HD], F32R, isOutput=False)
    wk = nc.declare_dram_parameter("wk", [C, HD], F32R, isOutput=False)
    wv = nc.declare_dram_parameter("wv", [C, HD], F32R, isOutput=False)
    wo = nc.declare_dram_parameter("wo", [HD, C], F32R, isOutput=False)
    outT = nc.declare_dram_parameter("outT", [C, L], F32, isOutput=True)

    with tile.TileContext(nc) as tc:
        with tc.tile_pool(name="sb", bufs=1) as sb, \
             tc.tile_pool(name="es_pool", bufs=2) as es_pool, \
             tc.tile_pool(name="st_pool", bufs=2) as st_pool, \
             tc.tile_pool(name="ps", bufs=2, space="PSUM") as ps, \
             tc.tile_pool(name="po", bufs=1, space="PSUM") as po, \
             tc.tile_pool(name="pr", bufs=1, space="PSUM") as pr:

            # ---- load inputs ------------------------------------------------
            xT_sb = sb.tile([128, KT, L], F32R, tag="xT")
            for k in range(KT):
                nc.sync.dma_start(xT_sb[:, k, :], xT[k * 128:(k + 1) * 128, :])

            wq_sb = sb.tile([128, KT, HD], F32R, tag="wq")
            wk_sb = sb.tile([128, KT, HD], F32R, tag="wk")
            wv_sb = sb.tile([128, KT, HD], F32R, tag="wv")
            for w_dram, w_sb in ((wq, wq_sb), (wk, wk_sb), (wv, wv_sb)):
                nc.sync.dma_start(
                    w_sb[:, :, :],
                    w_dram.rearrange("(k p) c -> p k c", p=128),
                )
            wo_sb = sb.tile([128, 2, C], F32R, tag="wo")
            nc.sync.dma_start(wo_sb[:, :, :], wo.rearrange("(k p) c -> p k c", p=128))

            ones_f = sb.tile([128, 64], F32, tag="ones_f")
            nc.vector.memset(ones_f[:], 1.0)
            ones_r = sb.tile([1, 64], F32R, tag="ones_r")
            nc.vector.tensor_copy(ones_r[:], ones_f[0:1, :])

            # ---- projections ------------------------------------------------
            # Q^T, K^T: [dh(2x128 part), i] ; head pair m holds heads 2m, 2m+1
            qT_sb = sb.tile([128, 2, L], F32R, tag="qT")
            kT_sb = sb.tile([128, 2, L], F32R, tag="kT")
            for w_sb, t_sb in ((wq_sb, qT_sb), (wk_sb, kT_sb)):
                for m in range(2):
                    for n in range(4):
                        p = ps.tile([128, 1024], F32, tag="s")
                        acc = p[:, 0:512]
                        for k in range(KT):
                            nc.tensor.matmul(
                                acc,
                                w_sb[:, k, m * 128:(m + 1) * 128],
                                xT_sb[:, k, n * 512:(n + 1) * 512],
                                start=(k == 0), stop=(k == KT - 1),
                            )
                        nc.vector.tensor_copy(
                            t_sb[:, m, n * 512:(n + 1) * 512], acc)

            # V with ones column: v_sb[p, j_tile, h, 0:64]=V, [..., 64]=1
            v_sb = sb.tile([128, 16, H_LOC, D + 1], F32R, tag="v")
            nc.vector.tensor_copy(
                v_sb[:, :, :, D:D + 1],
                ones_f.rearrange("p (a b c) -> p a b c", a=16, b=4),
            )
            for it in range(16):
                p = po.tile([128, 1024], F32, tag="o")
                acc = p[:, 0:HD]
                for k in range(KT):
                    nc.tensor.matmul(
                        acc,
                        xT_sb[:, k, it * 128:(it + 1) * 128],
                        wv_sb[:, k, :],
                        start=(k == 0), stop=(k == KT - 1),
                    )
                nc.vector.tensor_copy(
                    v_sb[:, it, :, 0:D],
                    acc.rearrange("p (h d) -> p h d", h=H_LOC),
                )

            # ---- attention --------------------------------------------------
            oT_sb = sb.tile([128, 2, L], F32R, tag="oT")
            recip = sb.tile([1, 1024], F32R, tag="recip")

            for h in range(H_LOC):
                m, r0 = divmod(h, 2)
                r0 *= 64
                for ih in range(2):  # i in halves of 1024
                    i0 = ih * 1024
                    o_full = po.tile([128, 1024], F32, tag="o", name="o_ps")
                    o_ps = o_full[0:65, :]
                    for j in range(16):
                        s_ps = ps.tile([128, 1024], F32, tag="s")
                        for n in range(2):
                            nc.tensor.matmul(
                                s_ps[:, n * 512:(n + 1) * 512],
                                kT_sb[r0:r0 + 64, m, j * 128:(j + 1) * 128],
                                qT_sb[r0:r0 + 64, m, i0 + n * 512:i0 + (n + 1) * 512],
                                start=True, stop=True,
                            )
                        e_sb = es_pool.tile([128, 1024], F32R, tag="es")
                        nc.scalar.activation(e_sb[:], s_ps[:], AF.Exp, scale=SCALE2)
                        for n in range(2):
                            nc.tensor.matmul(
                                o_ps[:, n * 512:(n + 1) * 512],
                                v_sb[:, j, h, :],
                                e_sb[:, n * 512:(n + 1) * 512],
                                start=(j == 0), stop=(j == 15),
                            )
                    # normalize: O^T = O'[0:64] / O'[64]
                    with nc.allow_low_precision(reason="f32r rounding for matmul"):
                        nc.vector.reciprocal(recip[:], o_ps[64:65, :])
                    rep_ps = pr.tile([64, 1024], F32, tag="r")
                    for n in range(2):
                        nc.tensor.matmul(
                            rep_ps[:, n * 512:(n + 1) * 512],
                            ones_r[:],
                            recip[:, n * 512:(n + 1) * 512],
                            start=True, stop=True,
                        )
                    rep_sb = st_pool.tile([64, 1024], F32, tag="rep")
                    nc.vector.tensor_copy(rep_sb[:], rep_ps[:])
                    with nc.allow_low_precision(reason="f32r rounding for matmul"):
                        if r0 == 0:
                            nc.vector.tensor_mul(
                                oT_sb[0:64, m, i0:i0 + 1024],
                                o_ps[0:64, :], rep_sb[:])
                        else:
                            stage = st_pool.tile([64, 1024], F32R, tag="stage")
                            nc.vector.tensor_mul(stage[:], o_ps[0:64, :], rep_sb[:])
                            nc.sync.dma_start(
                                oT_sb[64:128, m, i0:i0 + 1024], stage[:])

            # ---- output projection -----------------------------------------
            for ct in range(8):
                for n in range(4):
                    p = ps.tile([128, 1024], F32, tag="s")
                    acc = p[:, 0:512]
                    for kk in range(2):
                        nc.tensor.matmul(
                            acc,
                            wo_sb[:, kk, ct * 128:(ct + 1) * 128],
                            oT_sb[:, kk, n * 512:(n + 1) * 512],
                            start=(kk == 0), stop=(kk == 1),
                        )
                    ost = st_pool.tile([128, 512], F32, tag="ostage")
                    nc.vector.tensor_copy(ost[:], acc)
                    nc.sync.dma_start(
                        outT[ct * 128:(ct + 1) * 128, n * 512:(n + 1) * 512], ost[:])

    nc.compile()
    return nc


def kernel(x, Wq, Wk, Wv, Wo, bo):
    x = np.asarray(x, dtype=np.float32)
    Wq = np.asarray(Wq, dtype=np.float32)
    Wk = np.asarray(Wk, dtype=np.float32)
    Wv = np.asarray(Wv, dtype=np.float32)
    Wo = np.asarray(Wo, dtype=np.float32)
    bo = np.asarray(bo, dtype=np.float32)

    if "nc" not in _cache:
        _cache["nc"] = _build()
    nc = _cache["nc"]

    xTs = [np.ascontiguousarray(x[b].T) for b in range(B)]
    in_maps = []
    for core in range(8):
        b, hg = divmod(core, 4)
        sl = slice(hg * HD, (hg + 1) * HD)
        in_maps.append({
            "xT": xTs[b],
            "wq": np.ascontiguousarray(Wq[:, sl]),
            "wk": np.ascontiguousarray(Wk[:, sl]),
            "wv": np.ascontiguousarray(Wv[:, sl]),
            "wo": np.ascontiguousarray(Wo[sl, :]),
        })

    res = run_bass_kernel_spmd(nc, in_maps, core_ids=list(range(8)))
    out = np.empty((B, L, C), dtype=np.float32)
    for b in range(B):
        acc = res.results[4 * b]["outT"]
        for hg in range(1, 4):
            acc = acc + res.results[4 * b + hg]["outT"]
        out[b] = acc.T + bo
    return out
